# revision 1
# baseline (speedup 1.0000x reference)
"""Trainium2 Bass kernel for the ConvMod problem:

    Y1 = valid 2x2 cross-correlation(X, W)    # [4095, 4095]
    Y2 = transposed-conv(Y1, W)               # [4096, 4096]

The composite equals, in the interior, a 3x3 convolution of X with
K = corr(W, W), plus boundary corrections coming from the clipping of
Y1's domain:

    Y2 = Conv3x3_zeropad(X; K) - E_row - E_col + E_both

  E_row[0, q]    = sum_{b,d} W[1,b] W[1,d] Xpad[0,    q+b-d]
  E_row[H-1, q]  = sum_{b,d} W[0,b] W[0,d] Xpad[H-1,  q+b-d]
  E_col[p, 0]    = sum_{a,c} W[a,1] W[c,1] Xpad[p+a-c, 0]
  E_col[p, L-1]  = sum_{a,c} W[a,0] W[c,0] Xpad[p+a-c, L-1]
  E_both (corners): (0,0): W[1,1]^2 X[0,0]; (0,L-1): W[1,0]^2 X[0,L-1];
                    (H-1,0): W[0,1]^2 X[H-1,0]; (H-1,L-1): W[0,0]^2 X[...].

Distribution: data-parallel over rows across 8 cores; each core gets a
[514, 4096] row slab of X with a 1-row halo on each side (zero-padded at
the global edges), plus per-core stationary band matrices, and produces
its [512, 4096] slice of Y2.  No collectives.

On-device: rows live on SBUF partitions, columns on the free axis.  For a
block of M output rows we load an X tile of Kin = M+2 rows and run, per
512-column chunk, three TensorE matmuls with tridiagonal stationary band
matrices (one per column offset v in {-1,0,+1}; the column shift is
applied on the moving operand's free-axis slice), all accumulating into
one PSUM bank.  N=1 correction matmuls fix output columns 0 and L-1.
Row-boundary corrections are baked into the per-core stationary matrices.
PSUM is evacuated to SBUF alternately on ScalarE/VectorE and DMA'd out.
Matmuls run as float32r (input data is fp32 bit-identical; the PE runs
its fast reduced-precision fp32 path; accumulation is fp32).
"""

import numpy as np

import concourse.bass as bass
from concourse import bacc
import concourse.mybir as mybir
from concourse.tile import TileContext
from concourse.bass_utils import run_bass_kernel_spmd

H = 4096
L = 4096
LEXT = 4096 + 6            # slab columns: X row + [X0, 0, X0, XL, 0, XL] staging
NCORES = 8
RPC = H // NCORES          # output rows per core: 512
SLAB = RPC + 2             # input slab rows per core (1-row halo each side)
BLOCK_MS = [126, 126, 126, 126, 8]
BLOCK_STARTS = [0, 126, 252, 378, 504]
CHUNK = 512
NCH = L // CHUNK
WPAD_K = 128
WPAD_M = 126
NMATS = 15                 # 3 sets x 5 matrices
F32 = mybir.dt.float32
F32R = mybir.dt.float32r


# ----------------------------------------------------------------------------
# Host-side stationary-matrix construction
# ----------------------------------------------------------------------------

def _make_taps(W):
    W = np.asarray(W, dtype=np.float64)
    K = np.zeros((3, 3))
    for a in range(2):
        for b in range(2):
            for c in range(2):
                for d in range(2):
                    K[a - c + 1, b - d + 1] += W[a, b] * W[c, d]
    rowtop = np.zeros(3)
    rowbot = np.zeros(3)
    for b in range(2):
        for d in range(2):
            rowtop[b - d + 1] += W[1, b] * W[1, d]
            rowbot[b - d + 1] += W[0, b] * W[0, d]
    col0 = np.zeros(3)
    colL = np.zeros(3)
    for a in range(2):
        for c in range(2):
            col0[a - c + 1] += W[a, 1] * W[c, 1]
            colL[a - c + 1] += W[a, 0] * W[c, 0]
    corners = {
        (0, 0): W[1, 1] ** 2,
        (0, 1): W[1, 0] ** 2,
        (1, 0): W[0, 1] ** 2,
        (1, 1): W[0, 0] ** 2,
    }
    return K, rowtop, rowbot, col0, colL, corners


def _build_block_mats(W, M, first_row_global, last_row_global):
    """[5, M+2, M]: bands for v=-1,0,+1 then negated C0, C_L corrections."""
    K3, rowtop, rowbot, col0, colL, corners = _make_taps(W)
    Kin = M + 2
    mats = np.zeros((5, Kin, M))
    for m in range(M):
        for u in (-1, 0, 1):
            k = m + 1 + u
            for vi, v in enumerate((-1, 0, 1)):
                mats[vi, k, m] = K3[u + 1, v + 1]
            mats[3, k, m] = -col0[u + 1]
            mats[4, k, m] = -colL[u + 1]
    if first_row_global:
        for vi, v in enumerate((-1, 0, 1)):
            mats[vi, 1, 0] -= rowtop[v + 1]
        mats[3, 1, 0] += corners[(0, 0)]
        mats[4, 1, 0] += corners[(0, 1)]
    if last_row_global:
        m = M - 1
        for vi, v in enumerate((-1, 0, 1)):
            mats[vi, m + 1, m] -= rowbot[v + 1]
        mats[3, m + 1, m] += corners[(1, 0)]
        mats[4, m + 1, m] += corners[(1, 1)]
    return mats


def _build_wstack(W, core):
    """Per-core stationary stack [128, 15*126] (k-major, clean 2D DMA)."""
    out = np.zeros((WPAD_K, 3, 5, WPAD_M), dtype=np.float32)
    b0 = _build_block_mats(W, 126, core == 0, False)
    mid = _build_block_mats(W, 126, False, False)
    b4 = _build_block_mats(W, 8, False, core == NCORES - 1)
    for w in range(5):
        out[:128, 0, w, :126] = b0[w].astype(np.float32)
        out[:128, 1, w, :126] = mid[w].astype(np.float32)
        out[:10, 2, w, :8] = b4[w].astype(np.float32)
    return out.reshape(WPAD_K, NMATS * WPAD_M)


def _make_slabs(X):
    X = np.ascontiguousarray(np.asarray(X, dtype=np.float32))
    slabs = np.zeros((NCORES, SLAB, LEXT), dtype=np.float32)
    for c in range(NCORES):
        lo = c * RPC - 1
        hi = c * RPC + RPC + 1
        src_lo = max(0, lo)
        src_hi = min(H, hi)
        slabs[c, src_lo - lo : src_hi - lo, :L] = X[src_lo:src_hi, :]
    # staging columns for the N=2 edge-fix matmuls (PSUM writes must be
    # 8B-aligned with even N, so single-column terms are expressed as
    # [col, 0] / [0, col] pairs)
    slabs[:, :, L + 0] = slabs[:, :, 0]
    slabs[:, :, L + 2] = slabs[:, :, 0]
    slabs[:, :, L + 3] = slabs[:, :, L - 1]
    slabs[:, :, L + 5] = slabs[:, :, L - 1]
    return slabs


# ----------------------------------------------------------------------------
# Device program (SPMD; identical instruction stream on all 8 cores)
# ----------------------------------------------------------------------------

def build_nc(compile=True):
    nc = bacc.Bacc()
    x_d = nc.declare_dram_parameter("xslab", [SLAB, LEXT], F32R, isOutput=False)
    w_d = nc.declare_dram_parameter("wstack", [WPAD_K, NMATS * WPAD_M], F32R, isOutput=False)
    y_d = nc.declare_dram_parameter("y", [RPC, L], F32, isOutput=True)

    with TileContext(nc) as tc:
        with (
            tc.tile_pool(name="wp", bufs=1) as wp,
            tc.tile_pool(name="xp", bufs=5) as xp,
            tc.tile_pool(name="yp", bufs=4) as yp,
            tc.tile_pool(name="pp", bufs=8, space="PSUM") as pp,
        ):
            wsb = wp.tile([WPAD_K, NMATS * WPAD_M], F32R, name="wsb")
            # set 0 first (the only stationary set the first block needs);
            # sets 1/2 are issued after block 0's load pieces below
            nc.scalar.dma_start(
                out=wsb[:, 0 : 5 * WPAD_M], in_=w_d[:, 0 : 5 * WPAD_M]
            )

            for b in (0, 1, 4, 2, 3):
                M, s = BLOCK_MS[b], BLOCK_STARTS[b]
                Kin = M + 2
                si = 0 if b == 0 else (1 if b < 4 else 2)

                xt = xp.tile([128, LEXT], F32R, name=f"xt{b}", tag="xt")
                # column-split pieces so chunk-0 matmuls start after ~1/4 of
                # the tile has landed; the tiny staging-column piece goes
                # first (the chunk-0 edge matmuls read it)
                nc.scalar.dma_start(
                    out=xt[:Kin, L:LEXT], in_=x_d[s : s + Kin, L:LEXT]
                )
                for pi, (a, b_) in enumerate(
                    ((0, 1024), (1024, 2048), (2048, 3072), (3072, L))
                ):
                    dma_eng = nc.sync if pi % 2 == 0 else nc.scalar
                    dma_eng.dma_start(
                        out=xt[:Kin, a:b_], in_=x_d[s : s + Kin, a:b_]
                    )
                if b == 0:
                    # remaining stationary sets, needed from block 2 onward
                    nc.sync.dma_start(
                        out=wsb[:, 5 * WPAD_M :], in_=w_d[:, 5 * WPAD_M :]
                    )
                yt = yp.tile([128, L], F32, name=f"yt{b}", tag="yt")

                def wm(wi):
                    base = (si * 5 + wi) * WPAD_M
                    return wsb[0:Kin, base : base + M]

                def xr(c0, n):
                    return xt[0:Kin, c0 : c0 + n]

                pts = [
                    pp.tile([128, CHUNK], F32, name=f"pt{b}_{q}", tag="pt")
                    for q in range(NCH)
                ]

                # band v=0: full-width first touch per bank (start=True
                # clears the bank; partial-width bands then accumulate)
                for q in range(NCH):
                    nc.tensor.matmul(
                        pts[q][0:M, 0:CHUNK], wm(1), xr(q * CHUNK, CHUNK),
                        start=True, stop=False,
                    )
                # band v=-1 (psum writes must be 8B-aligned, even N:
                # chunk 0 covers [2:512); cols 0-1 are fixed below)
                nc.tensor.matmul(
                    pts[0][0:M, 2:CHUNK], wm(0), xr(1, CHUNK - 2),
                    start=False, stop=False,
                )
                for q in range(1, NCH):
                    nc.tensor.matmul(
                        pts[q][0:M, 0:CHUNK], wm(0), xr(q * CHUNK - 1, CHUNK),
                        start=False, stop=False,
                    )
                # band v=+1; chunk 0 first so its evacuation starts early
                nc.tensor.matmul(
                    pts[0][0:M, 0:CHUNK], wm(2), xr(1, CHUNK),
                    start=False, stop=False,
                )
                # left-edge fixes via N=2 matmuls on the [X0, 0, X0] staging
                # cols: col 0 += C0 . X0 ; col 1 += Band_-1 . X0
                nc.tensor.matmul(
                    pts[0][0:M, 0:2], wm(3), xr(L, 2), start=False, stop=False
                )
                nc.tensor.matmul(
                    pts[0][0:M, 0:2], wm(0), xr(L + 1, 2), start=False, stop=True
                )
                for q in range(1, NCH - 1):
                    nc.tensor.matmul(
                        pts[q][0:M, 0:CHUNK], wm(2), xr(q * CHUNK + 1, CHUNK),
                        start=False, stop=True,
                    )
                nc.tensor.matmul(
                    pts[NCH - 1][0:M, 0 : CHUNK - 2],
                    wm(2), xr((NCH - 1) * CHUNK + 1, CHUNK - 2),
                    start=False, stop=False,
                )
                # right-edge fixes on [XL, 0, XL]: col 510 += Band_+1 . XL ;
                # col 511 += C_L . XL
                nc.tensor.matmul(
                    pts[NCH - 1][0:M, CHUNK - 2 : CHUNK], wm(2), xr(L + 3, 2),
                    start=False, stop=False,
                )
                nc.tensor.matmul(
                    pts[NCH - 1][0:M, CHUNK - 2 : CHUNK], wm(4), xr(L + 4, 2),
                    start=False, stop=True,
                )

                # evacuate PSUM -> SBUF on two engines, then DMA out
                for q in range(NCH):
                    src = pts[q][0:M, 0:CHUNK]
                    dst = yt[0:M, q * CHUNK : (q + 1) * CHUNK]
                    if q in (0, 4, 6):
                        nc.scalar.copy(dst, src)
                    else:
                        nc.vector.tensor_copy(dst, src)
                # stores on SWDGE (GpSimd issue path is idle), in pieces
                # so they drain while later chunks still compute
                for a in range(0, L, 1024):
                    nc.gpsimd.dma_start(
                        out=y_d[s : s + M, a : a + 1024],
                        in_=yt[0:M, a : a + 1024],
                    )
    if compile:
        nc.compile()
    return nc


_NC_CACHE = None


def _get_nc():
    global _NC_CACHE
    if _NC_CACHE is None:
        _NC_CACHE = build_nc()
    return _NC_CACHE


def _run(X, W, trace=False, **spmd_kwargs):
    slabs = _make_slabs(X)
    in_maps = []
    for c in range(NCORES):
        in_maps.append(
            {"xslab": slabs[c], "wstack": _build_wstack(W, c)}
        )
    res = run_bass_kernel_spmd(
        _get_nc(), in_maps, core_ids=list(range(NCORES)), trace=trace, **spmd_kwargs
    )
    Y = np.concatenate([res.results[c]["y"] for c in range(NCORES)], axis=0)
    return Y, res


def kernel(X, W):
    Y, _ = _run(X, W)
    return Y



# revision 2
# speedup vs baseline: 1.4862x; 1.4862x over previous
"""Trainium2 Bass kernel for the ConvMod problem:

    Y1 = valid 2x2 cross-correlation(X, W)    # [4095, 4095]
    Y2 = transposed-conv(Y1, W)               # [4096, 4096]

The composite equals, in the interior, a 3x3 convolution of X with
K = corr(W, W), plus boundary corrections from the clipping of Y1's
domain (see _make_taps).

Distribution: data-parallel over rows across 8 cores; each core gets a
[514, 4104] fp16 row slab of X with a 1-row halo on each side, plus
per-core stationary band matrices, and produces its [512, 4096] slice
of Y2.  No collectives.  All HBM I/O is fp16 (the 2e-2 rel-err budget
has ~20x margin); PSUM accumulation is fp32.

Per core the 512 output rows split into 4 blocks of M=126 rows done as
tridiagonal band matmuls (3 column-offset passes over 4096 cols each,
PSUM-accumulated per 1024-col pair tile), plus an 8-row tail computed
in a column-folded layout [12 groups x 342 cols packed on partitions]
so its 3 band passes cost 342 moving columns instead of 4096.  Edge
corrections for output columns 0 / 4095 are N=2 matmuls on staging
columns; row-boundary corrections are baked into the per-core
stationary data (SPMD: same program, different data per core).

Engine roles: sync = input DMA (left halves + small tensors),
scalar = input DMA (right halves) + 2 PSUM evacuations per block,
vector = 2 evacuations per block + tail evac, gpsimd = output DMA
(SWDGE), tensor = matmuls only.  A few warmup matmuls on a zeroed
tile at t=0 climb the PE p-state ramp (0.65 -> 2.4 GHz) while the
first input DMA is in flight.
"""

import numpy as np

import concourse.bass as bass
from concourse import bacc
import concourse.mybir as mybir
from concourse.tile import TileContext
from concourse.bass_utils import run_bass_kernel_spmd

H = 4096
L = 4096
NCORES = 8
RPC = H // NCORES          # output rows per core: 512
SLAB = RPC + 2             # input slab rows per core (1-row halo each side)
STG = 6                    # staging cols at front: [X0, 0, 0, XL, 0, 0]
LEXT = STG + L + 2         # + 2 trailing zero cols (right pad for v=+1)
M = 126                    # output rows per main block
NBLK = 4                   # main blocks per core (4*126 = 504 rows)
MAIN = NBLK * M            # 504
PAIR = 1024                # psum pair-tile columns (2 banks)
NPAIR = L // PAIR          # 4
WPAD_M = 126
NSETS = 2                  # stationary sets: 0 = block 0, 1 = blocks 1..3
# tail: rows 504..511 in column-folded layout
TG = 12                    # groups
TGW = 342                  # cols per group (12*342 = 4104 >= 4096)
TR = 10                    # input rows for the tail (slab rows 504..513)
TM = 8                     # tail output rows
TKIN = TG * TR             # 120 moving partitions
TMOUT = TG * TM            # 96 output partitions
TXF = TGW + 2 + 4          # xtail free size: 344 window + 4 staging
NWARM = 6
F32 = mybir.dt.float32
F16 = mybir.dt.float16


# ----------------------------------------------------------------------------
# Host-side stationary-matrix construction
# ----------------------------------------------------------------------------

def _make_taps(W):
    W = np.asarray(W, dtype=np.float64)
    K = np.zeros((3, 3))
    for a in range(2):
        for b in range(2):
            for c in range(2):
                for d in range(2):
                    K[a - c + 1, b - d + 1] += W[a, b] * W[c, d]
    rowtop = np.zeros(3)
    rowbot = np.zeros(3)
    for b in range(2):
        for d in range(2):
            rowtop[b - d + 1] += W[1, b] * W[1, d]
            rowbot[b - d + 1] += W[0, b] * W[0, d]
    col0 = np.zeros(3)
    colL = np.zeros(3)
    for a in range(2):
        for c in range(2):
            col0[a - c + 1] += W[a, 1] * W[c, 1]
            colL[a - c + 1] += W[a, 0] * W[c, 0]
    corners = {
        (0, 0): W[1, 1] ** 2,
        (0, 1): W[1, 0] ** 2,
        (1, 0): W[0, 1] ** 2,
        (1, 1): W[0, 0] ** 2,
    }
    return K, rowtop, rowbot, col0, colL, corners


def _build_block_mats(W, Mb, first_row_global, last_row_global):
    """[5, Mb+2, Mb]: bands for v=-1,0,+1 then negated C0, C_L corrections."""
    K3, rowtop, rowbot, col0, colL, corners = _make_taps(W)
    Kin = Mb + 2
    mats = np.zeros((5, Kin, Mb))
    for m in range(Mb):
        for u in (-1, 0, 1):
            k = m + 1 + u
            for vi, v in enumerate((-1, 0, 1)):
                mats[vi, k, m] = K3[u + 1, v + 1]
            mats[3, k, m] = -col0[u + 1]
            mats[4, k, m] = -colL[u + 1]
    if first_row_global:
        for vi, v in enumerate((-1, 0, 1)):
            mats[vi, 1, 0] -= rowtop[v + 1]
        mats[3, 1, 0] += corners[(0, 0)]
        mats[4, 1, 0] += corners[(0, 1)]
    if last_row_global:
        m = Mb - 1
        for vi, v in enumerate((-1, 0, 1)):
            mats[vi, m + 1, m] -= rowbot[v + 1]
        mats[3, m + 1, m] += corners[(1, 0)]
        mats[4, m + 1, m] += corners[(1, 1)]
    return mats


def _build_wstack(W, core):
    """Per-core stationary stack [128, 10*126] fp16 (set-major, k-major)."""
    out = np.zeros((128, NSETS, 5, WPAD_M), dtype=np.float16)
    b0 = _build_block_mats(W, M, core == 0, False)
    mid = _build_block_mats(W, M, False, False)
    for w in range(5):
        out[:128, 0, w, :M] = b0[w].astype(np.float16)
        out[:128, 1, w, :M] = mid[w].astype(np.float16)
    return out.reshape(128, NSETS * 5 * WPAD_M)


def _build_wtail(W, core):
    """Tail stationary [120, 5*96] fp16: folded bands + SL + SR.

    S_w[g*TR + r, g*TM + m] = b4[w, r, m] for the 3 bands; SL only at
    g=0, SR only at g=TG-1 (their staging data is zero elsewhere, but
    zero coeffs keep it safe anyway)."""
    b4 = _build_block_mats(W, TM, False, core == NCORES - 1)  # [5, 10, 8]
    out = np.zeros((TKIN, 5, TMOUT), dtype=np.float16)
    for w in range(5):
        for g in range(TG):
            if w == 3 and g != 0:
                continue
            if w == 4 and g != TG - 1:
                continue
            out[g * TR : g * TR + TR, w, g * TM : g * TM + TM] = b4[w].astype(
                np.float16
            )
    return out.reshape(TKIN, 5 * TMOUT)


def _make_slabs(X16):
    """[8, SLAB, LEXT] fp16 slabs: staging cols 0..5 then X then 2 zero."""
    slabs = np.zeros((NCORES, SLAB, LEXT), dtype=np.float16)
    for c in range(NCORES):
        lo = c * RPC - 1
        hi = c * RPC + RPC + 1
        src_lo = max(0, lo)
        src_hi = min(H, hi)
        slabs[c, src_lo - lo : src_hi - lo, STG : STG + L] = X16[src_lo:src_hi, :]
    slabs[:, :, 0] = slabs[:, :, STG]          # X0
    slabs[:, :, 3] = slabs[:, :, STG + L - 1]  # XL
    return slabs


def _make_xtail(X16):
    """[8, TKIN, TXF] fp16 folded tail input, partition p = g*TR + r."""
    xt = np.zeros((NCORES, TKIN, TXF), dtype=np.float16)
    for c in range(NCORES):
        for r in range(TR):
            gr = c * RPC + MAIN - 1 + r
            if gr >= H:
                continue
            row = X16[gr]
            for g in range(TG):
                j0 = g * TGW - 1
                a = max(0, j0)
                b = min(L, j0 + TGW + 2)
                if a < b:
                    xt[c, g * TR + r, a - j0 : b - j0] = row[a:b]
            xt[c, 0 * TR + r, TGW + 2] = row[0]       # SL staging [X0, 0]
            xt[c, (TG - 1) * TR + r, TGW + 5] = row[L - 1]  # SR staging [0, XL]
    return xt


# ----------------------------------------------------------------------------
# Device program (SPMD; identical instruction stream on all 8 cores)
# ----------------------------------------------------------------------------

def build_nc(compile=True):
    nc = bacc.Bacc()
    x_d = nc.declare_dram_parameter("xslab", [SLAB, LEXT], F16, isOutput=False)
    w_d = nc.declare_dram_parameter(
        "wstack", [128, NSETS * 5 * WPAD_M], F16, isOutput=False
    )
    xt_d = nc.declare_dram_parameter("xtail", [TKIN, TXF], F16, isOutput=False)
    wt_d = nc.declare_dram_parameter("wtail", [TKIN, 5 * TMOUT], F16, isOutput=False)
    y_d = nc.declare_dram_parameter("y", [MAIN, L], F16, isOutput=True)
    yt_d = nc.declare_dram_parameter("ytail", [TMOUT, TGW], F16, isOutput=True)

    with TileContext(nc) as tc:
        with (
            tc.tile_pool(name="wp", bufs=1) as wp,
            tc.tile_pool(name="xp", bufs=4) as xp,
            tc.tile_pool(name="yp", bufs=4) as yp,
            tc.tile_pool(name="pp", bufs=3, space="PSUM") as pp,
            tc.tile_pool(name="pt", bufs=1, space="PSUM") as ppt,
            tc.tile_pool(name="pw", bufs=1, space="PSUM") as ppw,
        ):
            wsb = wp.tile([128, NSETS * 5 * WPAD_M], F16, name="wsb")
            wtsb = wp.tile([TKIN, 5 * TMOUT], F16, name="wtsb")
            xtsb = wp.tile([TKIN, TXF], F16, name="xtsb")
            wz = wp.tile([128, 512], F16, name="wz")
            ytlsb = wp.tile([TMOUT, TGW], F16, name="ytlsb")

            # -- input DMA triggers, all up front (queues stream ahead) --
            # sync queue: tail tensors then left halves
            nc.sync.dma_start(out=xtsb[:, :], in_=xt_d[:, :])
            nc.sync.dma_start(out=wtsb[:, :], in_=wt_d[:, :])
            # scalar queue: stationary stack then right halves
            nc.scalar.dma_start(out=wsb[:, :], in_=w_d[:, :])

            xts = []
            HALF = STG + 2048  # left half covers staging + X cols 0..2047
            for b in range(NBLK):
                s = b * M
                xt = xp.tile([128, LEXT], F16, name=f"xt{b}", tag="xt")
                xts.append(xt)
                nc.sync.dma_start(out=xt[:, 0:HALF], in_=x_d[s : s + 128, 0:HALF])
            for b in range(NBLK):
                s = b * M
                nc.scalar.dma_start(
                    out=xts[b][:, HALF:LEXT], in_=x_d[s : s + 128, HALF:LEXT]
                )

            # -- PE warmup on a zeroed tile (p-state ramp) --
            nc.gpsimd.memset(wz[:, :], 0.0)
            pw = ppw.tile([128, 512], F32, name="pwarm")
            for i in range(NWARM):
                nc.tensor.matmul(
                    pw[0:128, 0:512], wz[0:128, 0:128], wz[0:128, 0:512],
                    start=True, stop=True,
                )

            # -- tail (column-folded), first real work on the PE --
            def twm(w):
                return wtsb[0:TKIN, w * TMOUT : (w + 1) * TMOUT]

            ptl = ppt.tile([128, TGW], F32, name="ptail")
            nc.tensor.matmul(
                ptl[0:TMOUT, 0:TGW], twm(0), xtsb[0:TKIN, 0:TGW],
                start=True, stop=False,
            )
            nc.tensor.matmul(
                ptl[0:TMOUT, 0:TGW], twm(1), xtsb[0:TKIN, 1 : 1 + TGW],
                start=False, stop=False,
            )
            nc.tensor.matmul(
                ptl[0:TMOUT, 0:TGW], twm(2), xtsb[0:TKIN, 2 : 2 + TGW],
                start=False, stop=False,
            )
            nc.tensor.matmul(
                ptl[0:TMOUT, 0:2], twm(3), xtsb[0:TKIN, TGW + 2 : TGW + 4],
                start=False, stop=False,
            )
            nc.tensor.matmul(
                ptl[0:TMOUT, TGW - 10 : TGW - 8], twm(4),
                xtsb[0:TKIN, TGW + 4 : TGW + 6],
                start=False, stop=True,
            )
            nc.vector.tensor_copy(ytlsb[0:TMOUT, 0:TGW], ptl[0:TMOUT, 0:TGW])
            nc.gpsimd.dma_start(out=yt_d[:, :], in_=ytlsb[0:TMOUT, 0:TGW])

            # -- main blocks --
            for b in range(NBLK):
                s = b * M
                si = 0 if b == 0 else 1
                xt = xts[b]
                yt = yp.tile([128, L], F16, name=f"yt{b}", tag="yt")

                def wm(w):
                    base = (si * 5 + w) * WPAD_M
                    return wsb[0:128, base : base + M]

                def xr(c0, n):
                    # moving slice for X cols [c0, c0+n) (c0 may be -1 to
                    # touch the staged zero at tile col 5)
                    return xt[0:128, STG + c0 : STG + c0 + n]

                for k in range(NPAIR):
                    pt = pp.tile([128, PAIR], F32, name=f"pt{b}_{k}", tag="pt")
                    cA = k * PAIR
                    cB = cA + 512
                    # band v=0 (start clears each psum half)
                    nc.tensor.matmul(
                        pt[0:M, 0:512], wm(1), xr(cA, 512), start=True, stop=False
                    )
                    nc.tensor.matmul(
                        pt[0:M, 512:1024], wm(1), xr(cB, 512), start=True, stop=False
                    )
                    # band v=-1
                    nc.tensor.matmul(
                        pt[0:M, 0:512], wm(0), xr(cA - 1, 512),
                        start=False, stop=False,
                    )
                    nc.tensor.matmul(
                        pt[0:M, 512:1024], wm(0), xr(cB - 1, 512),
                        start=False, stop=False,
                    )
                    # band v=+1 (+ edge fixes on the outermost pairs)
                    nc.tensor.matmul(
                        pt[0:M, 0:512], wm(2), xr(cA + 1, 512),
                        start=False, stop=(k != 0),
                    )
                    if k == 0:
                        # col 0 fix: moving staging [X0, 0] at tile cols 0:2
                        nc.tensor.matmul(
                            pt[0:M, 0:2], wm(3), xt[0:128, 0:2],
                            start=False, stop=True,
                        )
                    nc.tensor.matmul(
                        pt[0:M, 512:1024], wm(2), xr(cB + 1, 512),
                        start=False, stop=(k != NPAIR - 1),
                    )
                    if k == NPAIR - 1:
                        # col L-1 fix: moving staging [0, XL] at tile cols 2:4
                        nc.tensor.matmul(
                            pt[0:M, 1022:1024], wm(4), xt[0:128, 2:4],
                            start=False, stop=True,
                        )
                    # evacuate pair to SBUF (fp32 -> fp16)
                    dst = yt[0:M, cA : cA + PAIR]
                    src = pt[0:M, 0:PAIR]
                    if k % 2 == 0:
                        nc.scalar.copy(dst, src)
                    else:
                        nc.vector.tensor_copy(dst, src)

                # output DMA on SWDGE; split the last block for tail latency
                if b < NBLK - 1:
                    nc.gpsimd.dma_start(
                        out=y_d[s : s + M, 0:L], in_=yt[0:M, 0:L]
                    )
                else:
                    nc.gpsimd.dma_start(
                        out=y_d[s : s + M, 0:2048], in_=yt[0:M, 0:2048]
                    )
                    nc.gpsimd.dma_start(
                        out=y_d[s : s + M, 2048:L], in_=yt[0:M, 2048:L]
                    )
    if compile:
        nc.compile()
    return nc


_NC_CACHE = None


def _get_nc():
    global _NC_CACHE
    if _NC_CACHE is None:
        _NC_CACHE = build_nc()
    return _NC_CACHE


def _run(X, W, trace=False, **spmd_kwargs):
    X16 = np.asarray(X, dtype=np.float16)
    slabs = _make_slabs(X16)
    xtails = _make_xtail(X16)
    in_maps = []
    for c in range(NCORES):
        in_maps.append(
            {
                "xslab": slabs[c],
                "wstack": _build_wstack(W, c),
                "xtail": xtails[c],
                "wtail": _build_wtail(W, c),
            }
        )
    res = run_bass_kernel_spmd(
        _get_nc(), in_maps, core_ids=list(range(NCORES)), trace=trace, **spmd_kwargs
    )
    Y = np.empty((H, L), dtype=np.float32)
    for c in range(NCORES):
        r0 = c * RPC
        Y[r0 : r0 + MAIN] = res.results[c]["y"].astype(np.float32)
        ytl = res.results[c]["ytail"].astype(np.float32)  # [96, 342]
        tail = ytl.reshape(TG, TM, TGW).transpose(1, 0, 2).reshape(TM, TG * TGW)
        Y[r0 + MAIN : r0 + RPC] = tail[:, :L]
    return Y, res


def kernel(X, W):
    Y, _ = _run(X, W)
    return Y


# revision 8
# speedup vs baseline: 1.5212x; 1.0236x over previous
"""Trainium2 Bass kernel for the ConvMod problem:

    Y1 = valid 2x2 cross-correlation(X, W)    # [4095, 4095]
    Y2 = transposed-conv(Y1, W)               # [4096, 4096]

The composite equals, in the interior, a 3x3 convolution of X with
K = corr(W, W), plus boundary corrections from the clipping of Y1's
domain (see _make_taps).

Distribution: data-parallel over rows across 8 cores; each core gets a
[514, 4104] fp16 row slab of X with a 1-row halo on each side, plus
per-core stationary band matrices, and produces its [512, 4096] slice
of Y2.  No collectives.  All HBM I/O is fp16 (the 2e-2 rel-err budget
has ~20x margin); PSUM accumulation is fp32.

Per core the 512 output rows split into 4 blocks of M=126 rows done as
tridiagonal band matmuls (3 column-offset passes over 4096 cols each,
PSUM-accumulated per 1024-col pair tile), plus an 8-row tail computed
in a column-folded layout [12 groups x 342 cols packed on partitions]
so its 3 band passes cost 342 moving columns instead of 4096.  Edge
corrections for output columns 0 / 4095 are N=2 matmuls on staging
columns; row-boundary corrections are baked into the per-core
stationary data (SPMD: same program, different data per core).

Engine roles: sync = input DMA (left halves + small tensors),
scalar = input DMA (right halves) + 2 PSUM evacuations per block,
vector = 2 evacuations per block + tail evac, gpsimd = output DMA
(SWDGE), tensor = matmuls only.  A few warmup matmuls on a zeroed
tile at t=0 climb the PE p-state ramp (0.65 -> 2.4 GHz) while the
first input DMA is in flight.
"""

import numpy as np

import concourse.bass as bass
from concourse import bacc
import concourse.mybir as mybir
from concourse.tile import TileContext
from concourse.bass_utils import run_bass_kernel_spmd

H = 4096
L = 4096
NCORES = 8
RPC = H // NCORES          # output rows per core: 512
SLAB = RPC + 2             # input slab rows per core (1-row halo each side)
STG = 6                    # staging cols at front: [X0, 0, 0, XL, 0, 0]
LEXT = STG + L + 2         # + 2 trailing zero cols (right pad for v=+1)
M = 126                    # output rows per main block
NBLK = 4                   # main blocks per core (4*126 = 504 rows)
MAIN = NBLK * M            # 504
PAIR = 1024                # psum pair-tile columns (2 banks)
NPAIR = L // PAIR          # 4
WPAD_M = 126
NSETS = 2                  # stationary sets: 0 = block 0, 1 = blocks 1..3
# tail: rows 504..511 in column-folded layout
TG = 12                    # groups
TGW = 342                  # cols per group (12*342 = 4104 >= 4096)
TR = 10                    # input rows for the tail (slab rows 504..513)
TM = 8                     # tail output rows
TKIN = TG * TR             # 120 moving partitions
TMOUT = TG * TM            # 96 output partitions
TXF = TGW + 2 + 4          # xtail free size: 344 window + 4 staging
NWARM = 10
F32 = mybir.dt.float32
F16 = mybir.dt.float16


# ----------------------------------------------------------------------------
# Host-side stationary-matrix construction
# ----------------------------------------------------------------------------

def _make_taps(W):
    W = np.asarray(W, dtype=np.float64)
    K = np.zeros((3, 3))
    for a in range(2):
        for b in range(2):
            for c in range(2):
                for d in range(2):
                    K[a - c + 1, b - d + 1] += W[a, b] * W[c, d]
    rowtop = np.zeros(3)
    rowbot = np.zeros(3)
    for b in range(2):
        for d in range(2):
            rowtop[b - d + 1] += W[1, b] * W[1, d]
            rowbot[b - d + 1] += W[0, b] * W[0, d]
    col0 = np.zeros(3)
    colL = np.zeros(3)
    for a in range(2):
        for c in range(2):
            col0[a - c + 1] += W[a, 1] * W[c, 1]
            colL[a - c + 1] += W[a, 0] * W[c, 0]
    corners = {
        (0, 0): W[1, 1] ** 2,
        (0, 1): W[1, 0] ** 2,
        (1, 0): W[0, 1] ** 2,
        (1, 1): W[0, 0] ** 2,
    }
    return K, rowtop, rowbot, col0, colL, corners


def _build_block_mats(W, Mb, first_row_global, last_row_global):
    """[5, Mb+2, Mb]: bands for v=-1,0,+1 then negated C0, C_L corrections."""
    K3, rowtop, rowbot, col0, colL, corners = _make_taps(W)
    Kin = Mb + 2
    mats = np.zeros((5, Kin, Mb))
    for m in range(Mb):
        for u in (-1, 0, 1):
            k = m + 1 + u
            for vi, v in enumerate((-1, 0, 1)):
                mats[vi, k, m] = K3[u + 1, v + 1]
            mats[3, k, m] = -col0[u + 1]
            mats[4, k, m] = -colL[u + 1]
    if first_row_global:
        for vi, v in enumerate((-1, 0, 1)):
            mats[vi, 1, 0] -= rowtop[v + 1]
        mats[3, 1, 0] += corners[(0, 0)]
        mats[4, 1, 0] += corners[(0, 1)]
    if last_row_global:
        m = Mb - 1
        for vi, v in enumerate((-1, 0, 1)):
            mats[vi, m + 1, m] -= rowbot[v + 1]
        mats[3, m + 1, m] += corners[(1, 0)]
        mats[4, m + 1, m] += corners[(1, 1)]
    return mats


def _build_wstack(W, core):
    """Per-core stationary stack [128, 10*126] fp16 (set-major, k-major)."""
    out = np.zeros((128, NSETS, 5, WPAD_M), dtype=np.float16)
    b0 = _build_block_mats(W, M, core == 0, False)
    mid = _build_block_mats(W, M, False, False)
    for w in range(5):
        out[:128, 0, w, :M] = b0[w].astype(np.float16)
        out[:128, 1, w, :M] = mid[w].astype(np.float16)
    return out.reshape(128, NSETS * 5 * WPAD_M)


def _build_wtail(W, core):
    """Tail stationary [120, 5*96] fp16: folded bands + SL + SR.

    S_w[g*TR + r, g*TM + m] = b4[w, r, m] for the 3 bands; SL only at
    g=0, SR only at g=TG-1 (their staging data is zero elsewhere, but
    zero coeffs keep it safe anyway)."""
    b4 = _build_block_mats(W, TM, False, core == NCORES - 1)  # [5, 10, 8]
    out = np.zeros((TKIN, 5, TMOUT), dtype=np.float16)
    for w in range(5):
        for g in range(TG):
            if w == 3 and g != 0:
                continue
            if w == 4 and g != TG - 1:
                continue
            out[g * TR : g * TR + TR, w, g * TM : g * TM + TM] = b4[w].astype(
                np.float16
            )
    return out.reshape(TKIN, 5 * TMOUT)


def _make_slabs(X16):
    """[8, SLAB, LEXT] fp16 slabs: staging cols 0..5 then X then 2 zero."""
    slabs = np.zeros((NCORES, SLAB, LEXT), dtype=np.float16)
    for c in range(NCORES):
        lo = c * RPC - 1
        hi = c * RPC + RPC + 1
        src_lo = max(0, lo)
        src_hi = min(H, hi)
        slabs[c, src_lo - lo : src_hi - lo, STG : STG + L] = X16[src_lo:src_hi, :]
    slabs[:, :, 0] = slabs[:, :, STG]          # X0
    slabs[:, :, 3] = slabs[:, :, STG + L - 1]  # XL
    return slabs


def _make_xtail(X16):
    """[8, TKIN, TXF] fp16 folded tail input, partition p = g*TR + r."""
    xt = np.zeros((NCORES, TKIN, TXF), dtype=np.float16)
    for c in range(NCORES):
        for r in range(TR):
            gr = c * RPC + MAIN - 1 + r
            if gr >= H:
                continue
            row = X16[gr]
            for g in range(TG):
                j0 = g * TGW - 1
                a = max(0, j0)
                b = min(L, j0 + TGW + 2)
                if a < b:
                    xt[c, g * TR + r, a - j0 : b - j0] = row[a:b]
            xt[c, 0 * TR + r, TGW + 2] = row[0]       # SL staging [X0, 0]
            xt[c, (TG - 1) * TR + r, TGW + 5] = row[L - 1]  # SR staging [0, XL]
    return xt


# ----------------------------------------------------------------------------
# Device program (SPMD; identical instruction stream on all 8 cores)
# ----------------------------------------------------------------------------

def build_nc(compile=True):
    nc = bacc.Bacc()
    x_d = nc.declare_dram_parameter("xslab", [SLAB, LEXT], F16, isOutput=False)
    w_d = nc.declare_dram_parameter(
        "wstack", [128, NSETS * 5 * WPAD_M], F16, isOutput=False
    )
    xt_d = nc.declare_dram_parameter("xtail", [TKIN, TXF], F16, isOutput=False)
    wt_d = nc.declare_dram_parameter("wtail", [TKIN, 5 * TMOUT], F16, isOutput=False)
    y_d = nc.declare_dram_parameter("y", [MAIN, L], F16, isOutput=True)
    yt_d = nc.declare_dram_parameter("ytail", [TMOUT, TGW], F16, isOutput=True)

    with TileContext(nc) as tc:
        with (
            tc.tile_pool(name="wp", bufs=1) as wp,
            tc.tile_pool(name="xp", bufs=4) as xp,
            tc.tile_pool(name="yp", bufs=4) as yp,
            tc.tile_pool(name="pp", bufs=3, space="PSUM") as pp,
            tc.tile_pool(name="pt", bufs=1, space="PSUM") as ppt,
            tc.tile_pool(name="pw", bufs=1, space="PSUM") as ppw,
        ):
            wsb = wp.tile([128, NSETS * 5 * WPAD_M], F16, name="wsb")
            wtsb = wp.tile([TKIN, 5 * TMOUT], F16, name="wtsb")
            xtsb = wp.tile([TKIN, TXF], F16, name="xtsb")
            wz = wp.tile([128, 512], F16, name="wz")
            ytlsb = wp.tile([TMOUT, TGW], F16, name="ytlsb")

            # -- input DMA triggers, all up front (queues stream ahead) --
            # gpsimd SWDGE queue: stationary stacks + folded tail input
            nc.gpsimd.dma_start(out=wsb[:, :], in_=w_d[:, :])
            nc.gpsimd.dma_start(out=xtsb[:, :], in_=xt_d[:, :])
            nc.gpsimd.dma_start(out=wtsb[:, :], in_=wt_d[:, :])

            xts = [
                xp.tile([128, LEXT], F16, name=f"xt{b}", tag="xt")
                for b in range(NBLK)
            ]
            # block 0 quartered across sync+scalar so pair-0 matmuls can
            # start as early as possible; blocks 1..3 as L/R halves
            Q = [0, STG + 1024, STG + 2048, STG + 3072, LEXT]
            nc.sync.dma_start(out=xts[0][:, Q[0] : Q[1]], in_=x_d[0:128, Q[0] : Q[1]])
            nc.scalar.dma_start(
                out=xts[0][:, Q[1] : Q[2]], in_=x_d[0:128, Q[1] : Q[2]]
            )
            nc.sync.dma_start(out=xts[0][:, Q[2] : Q[3]], in_=x_d[0:128, Q[2] : Q[3]])
            nc.scalar.dma_start(
                out=xts[0][:, Q[3] : Q[4]], in_=x_d[0:128, Q[3] : Q[4]]
            )
            HALF = STG + 2048  # left half covers staging + X cols 0..2047
            for b in range(1, NBLK):
                s = b * M
                nc.sync.dma_start(
                    out=xts[b][:, 0:HALF], in_=x_d[s : s + 128, 0:HALF]
                )
            for b in range(1, NBLK):
                s = b * M
                nc.scalar.dma_start(
                    out=xts[b][:, HALF:LEXT], in_=x_d[s : s + 128, HALF:LEXT]
                )

            # -- PE warmup (p-state ramp); results are discarded --
            nc.vector.memset(wz[:, :], 0.0)
            pw = ppw.tile([128, 512], F32, name="pwarm")
            for i in range(NWARM):
                nc.tensor.matmul(
                    pw[0:128, 0:512], wz[0:128, 0:128], wz[0:128, 0:512],
                    start=True, stop=True,
                )

            # -- tail (column-folded), first real work on the PE --
            def twm(w):
                return wtsb[0:TKIN, w * TMOUT : (w + 1) * TMOUT]

            ptl = ppt.tile([128, TGW], F32, name="ptail")
            nc.tensor.matmul(
                ptl[0:TMOUT, 0:TGW], twm(0), xtsb[0:TKIN, 0:TGW],
                start=True, stop=False,
            )
            nc.tensor.matmul(
                ptl[0:TMOUT, 0:TGW], twm(1), xtsb[0:TKIN, 1 : 1 + TGW],
                start=False, stop=False,
            )
            nc.tensor.matmul(
                ptl[0:TMOUT, 0:TGW], twm(2), xtsb[0:TKIN, 2 : 2 + TGW],
                start=False, stop=False,
            )
            nc.tensor.matmul(
                ptl[0:TMOUT, 0:2], twm(3), xtsb[0:TKIN, TGW + 2 : TGW + 4],
                start=False, stop=False,
            )
            nc.tensor.matmul(
                ptl[0:TMOUT, TGW - 10 : TGW - 8], twm(4),
                xtsb[0:TKIN, TGW + 4 : TGW + 6],
                start=False, stop=True,
            )
            nc.vector.tensor_copy(ytlsb[0:TMOUT, 0:TGW], ptl[0:TMOUT, 0:TGW])
            nc.gpsimd.dma_start(out=yt_d[:, :], in_=ytlsb[0:TMOUT, 0:TGW])

            # -- main blocks --
            for b in range(NBLK):
                s = b * M
                si = 0 if b == 0 else 1
                xt = xts[b]
                yt = yp.tile([128, L], F16, name=f"yt{b}", tag="yt")

                def wm(w):
                    base = (si * 5 + w) * WPAD_M
                    return wsb[0:128, base : base + M]

                def xr(c0, n):
                    # moving slice for X cols [c0, c0+n) (c0 may be -1 to
                    # touch the staged zero at tile col 5)
                    return xt[0:128, STG + c0 : STG + c0 + n]

                for k in range(NPAIR):
                    pt = pp.tile([128, PAIR], F32, name=f"pt{b}_{k}", tag="pt")
                    cA = k * PAIR
                    cB = cA + 512
                    # band v=0 (start clears each psum half)
                    nc.tensor.matmul(
                        pt[0:M, 0:512], wm(1), xr(cA, 512), start=True, stop=False
                    )
                    nc.tensor.matmul(
                        pt[0:M, 512:1024], wm(1), xr(cB, 512), start=True, stop=False
                    )
                    # band v=-1
                    nc.tensor.matmul(
                        pt[0:M, 0:512], wm(0), xr(cA - 1, 512),
                        start=False, stop=False,
                    )
                    nc.tensor.matmul(
                        pt[0:M, 512:1024], wm(0), xr(cB - 1, 512),
                        start=False, stop=False,
                    )
                    # band v=+1 (+ edge fixes on the outermost pairs)
                    nc.tensor.matmul(
                        pt[0:M, 0:512], wm(2), xr(cA + 1, 512),
                        start=False, stop=(k != 0),
                    )
                    if k == 0:
                        # col 0 fix: moving staging [X0, 0] at tile cols 0:2
                        nc.tensor.matmul(
                            pt[0:M, 0:2], wm(3), xt[0:128, 0:2],
                            start=False, stop=True,
                        )
                    nc.tensor.matmul(
                        pt[0:M, 512:1024], wm(2), xr(cB + 1, 512),
                        start=False, stop=(k != NPAIR - 1),
                    )
                    if k == NPAIR - 1:
                        # col L-1 fix: moving staging [0, XL] at tile cols 2:4
                        nc.tensor.matmul(
                            pt[0:M, 1022:1024], wm(4), xt[0:128, 2:4],
                            start=False, stop=True,
                        )
                    # evacuate pair to SBUF (fp32 -> fp16); the very last
                    # pair is split across both engines for tail latency
                    if b == NBLK - 1 and k == NPAIR - 1:
                        nc.scalar.copy(
                            yt[0:M, cA : cA + 512], pt[0:M, 0:512]
                        )
                        nc.vector.tensor_copy(
                            yt[0:M, cB : cB + 512], pt[0:M, 512:1024]
                        )
                    else:
                        dst = yt[0:M, cA : cA + PAIR]
                        src = pt[0:M, 0:PAIR]
                        if k % 2 == 0:
                            nc.scalar.copy(dst, src)
                        else:
                            nc.vector.tensor_copy(dst, src)
                    # stream the output out at half-block granularity on the
                    # vector HWDGE queue (idle after the early input loads);
                    # quarters at the very end for drain latency
                    if k == 1:
                        nc.gpsimd.dma_start(
                            out=y_d[s : s + M, 0:2048], in_=yt[0:M, 0:2048]
                        )
                    elif k == 3:
                        if b < NBLK - 1:
                            nc.gpsimd.dma_start(
                                out=y_d[s : s + M, 2048:L], in_=yt[0:M, 2048:L]
                            )
                        else:
                            nc.gpsimd.dma_start(
                                out=y_d[s : s + M, 2048:3072],
                                in_=yt[0:M, 2048:3072],
                            )
                            nc.gpsimd.dma_start(
                                out=y_d[s : s + M, 3072:L], in_=yt[0:M, 3072:L]
                            )
    if compile:
        nc.compile()
    return nc


_NC_CACHE = None


def _get_nc():
    global _NC_CACHE
    if _NC_CACHE is None:
        _NC_CACHE = build_nc()
    return _NC_CACHE


def _run(X, W, trace=False, **spmd_kwargs):
    X16 = np.asarray(X, dtype=np.float16)
    slabs = _make_slabs(X16)
    xtails = _make_xtail(X16)
    in_maps = []
    for c in range(NCORES):
        in_maps.append(
            {
                "xslab": slabs[c],
                "wstack": _build_wstack(W, c),
                "xtail": xtails[c],
                "wtail": _build_wtail(W, c),
            }
        )
    res = run_bass_kernel_spmd(
        _get_nc(), in_maps, core_ids=list(range(NCORES)), trace=trace, **spmd_kwargs
    )
    Y = np.empty((H, L), dtype=np.float32)
    for c in range(NCORES):
        r0 = c * RPC
        Y[r0 : r0 + MAIN] = res.results[c]["y"].astype(np.float32)
        ytl = res.results[c]["ytail"].astype(np.float32)  # [96, 342]
        tail = ytl.reshape(TG, TM, TGW).transpose(1, 0, 2).reshape(TM, TG * TGW)
        Y[r0 + MAIN : r0 + RPC] = tail[:, :L]
    return Y, res


def kernel(X, W):
    Y, _ = _run(X, W)
    return Y


# revision 11
# speedup vs baseline: 1.7234x; 1.1329x over previous
"""Trainium2 Bass kernel for the ConvMod problem:

    Y1 = valid 2x2 cross-correlation(X, W)    # [4095, 4095]
    Y2 = transposed-conv(Y1, W)               # [4096, 4096]

The composite equals, in the interior, a 3x3 convolution of X with
K = corr(W, W), plus boundary corrections from the clipping of Y1's
domain (see _make_taps).

Distribution: data-parallel over rows across 8 cores; each core gets a
[514, 4104] fp16 row slab of X with a 1-row halo on each side, plus
per-core stationary band matrices, and produces its [512, 4096] slice
of Y2.  No collectives.  All HBM I/O is fp16 (the 2e-2 rel-err budget
has ~20x margin); PSUM accumulation is fp32.

Per core the 512 output rows split into 4 blocks of M=126 rows done as
tridiagonal band matmuls (3 column-offset passes over 4096 cols each,
PSUM-accumulated per 1024-col pair tile), plus an 8-row tail computed
in a column-folded layout [12 groups x 342 cols packed on partitions]
so its 3 band passes cost 342 moving columns instead of 4096.  Edge
corrections for output columns 0 / 4095 are N=2 matmuls on staging
columns; row-boundary corrections are baked into the per-core
stationary data (SPMD: same program, different data per core).

Engine roles: sync = input DMA (left halves + small tensors),
scalar = input DMA (right halves) + 2 PSUM evacuations per block,
vector = 2 evacuations per block + tail evac, gpsimd = output DMA
(SWDGE), tensor = matmuls only.  A few warmup matmuls on a zeroed
tile at t=0 climb the PE p-state ramp (0.65 -> 2.4 GHz) while the
first input DMA is in flight.
"""

import numpy as np

import concourse.bass as bass
from concourse import bacc
import concourse.mybir as mybir
from concourse.tile import TileContext
from concourse.bass_utils import run_bass_kernel_spmd

H = 4096
L = 4096
NCORES = 8
RPC = H // NCORES          # output rows per core: 512
SLAB = RPC + 2             # input slab rows per core (1-row halo each side)
STG = 6                    # staging cols at front: [X0, 0, 0, XL, 0, 0]
LEXT = STG + L + 2         # + 2 trailing zero cols (right pad for v=+1)
M = 126                    # output rows per main block
NBLK = 4                   # main blocks per core (4*126 = 504 rows)
MAIN = NBLK * M            # 504
PAIR = 1024                # psum pair-tile columns (2 banks)
NPAIR = L // PAIR          # 4
WPAD_M = 126
NSETS = 2                  # stationary sets: 0 = block 0, 1 = blocks 1..3
# tail: rows 504..511 in column-folded layout
TG = 12                    # groups
TGW = 342                  # cols per group (12*342 = 4104 >= 4096)
TR = 10                    # input rows for the tail (slab rows 504..513)
TM = 8                     # tail output rows
TKIN = TG * TR             # 120 moving partitions
TMOUT = TG * TM            # 96 output partitions
TXF = TGW + 2 + 4          # xtail free size: 344 window + 4 staging
NWARM = 12
F32 = mybir.dt.float32
F16 = mybir.dt.float16


# ----------------------------------------------------------------------------
# Host-side stationary-matrix construction
# ----------------------------------------------------------------------------

def _make_taps(W):
    W = np.asarray(W, dtype=np.float64)
    K = np.zeros((3, 3))
    for a in range(2):
        for b in range(2):
            for c in range(2):
                for d in range(2):
                    K[a - c + 1, b - d + 1] += W[a, b] * W[c, d]
    rowtop = np.zeros(3)
    rowbot = np.zeros(3)
    for b in range(2):
        for d in range(2):
            rowtop[b - d + 1] += W[1, b] * W[1, d]
            rowbot[b - d + 1] += W[0, b] * W[0, d]
    col0 = np.zeros(3)
    colL = np.zeros(3)
    for a in range(2):
        for c in range(2):
            col0[a - c + 1] += W[a, 1] * W[c, 1]
            colL[a - c + 1] += W[a, 0] * W[c, 0]
    corners = {
        (0, 0): W[1, 1] ** 2,
        (0, 1): W[1, 0] ** 2,
        (1, 0): W[0, 1] ** 2,
        (1, 1): W[0, 0] ** 2,
    }
    return K, rowtop, rowbot, col0, colL, corners


def _build_block_mats(W, Mb, first_row_global, last_row_global):
    """[5, Mb+2, Mb]: bands for v=-1,0,+1 then negated C0, C_L corrections."""
    K3, rowtop, rowbot, col0, colL, corners = _make_taps(W)
    Kin = Mb + 2
    mats = np.zeros((5, Kin, Mb))
    for m in range(Mb):
        for u in (-1, 0, 1):
            k = m + 1 + u
            for vi, v in enumerate((-1, 0, 1)):
                mats[vi, k, m] = K3[u + 1, v + 1]
            mats[3, k, m] = -col0[u + 1]
            mats[4, k, m] = -colL[u + 1]
    if first_row_global:
        for vi, v in enumerate((-1, 0, 1)):
            mats[vi, 1, 0] -= rowtop[v + 1]
        mats[3, 1, 0] += corners[(0, 0)]
        mats[4, 1, 0] += corners[(0, 1)]
    if last_row_global:
        m = Mb - 1
        for vi, v in enumerate((-1, 0, 1)):
            mats[vi, m + 1, m] -= rowbot[v + 1]
        mats[3, m + 1, m] += corners[(1, 0)]
        mats[4, m + 1, m] += corners[(1, 1)]
    return mats


def _build_wstack(W, core):
    """Per-core stationary stack [128, 10*126] fp16 (set-major, k-major)."""
    out = np.zeros((128, NSETS, 5, WPAD_M), dtype=np.float16)
    b0 = _build_block_mats(W, M, core == 0, False)
    mid = _build_block_mats(W, M, False, False)
    for w in range(5):
        out[:128, 0, w, :M] = b0[w].astype(np.float16)
        out[:128, 1, w, :M] = mid[w].astype(np.float16)
    return out.reshape(128, NSETS * 5 * WPAD_M)


# packed [128, SM_TOT] layout: wstack | wtail | xtail (fat DMA lines)
SM_WS = NSETS * 5 * WPAD_M          # 1260
SM_WT = SM_WS + 5 * TMOUT           # 1740
SM_TOT = SM_WT + TXF                # 2088


def _build_smalls(W, core, xtail_c):
    out = np.zeros((128, SM_TOT), dtype=np.float16)
    out[:, :SM_WS] = _build_wstack(W, core)
    out[:TKIN, SM_WS:SM_WT] = _build_wtail(W, core)
    out[:TKIN, SM_WT:] = xtail_c
    return out


def _build_wtail(W, core):
    """Tail stationary [120, 5*96] fp16: folded bands + SL + SR.

    S_w[g*TR + r, g*TM + m] = b4[w, r, m] for the 3 bands; SL only at
    g=0, SR only at g=TG-1 (their staging data is zero elsewhere, but
    zero coeffs keep it safe anyway)."""
    b4 = _build_block_mats(W, TM, False, core == NCORES - 1)  # [5, 10, 8]
    out = np.zeros((TKIN, 5, TMOUT), dtype=np.float16)
    for w in range(5):
        for g in range(TG):
            if w == 3 and g != 0:
                continue
            if w == 4 and g != TG - 1:
                continue
            out[g * TR : g * TR + TR, w, g * TM : g * TM + TM] = b4[w].astype(
                np.float16
            )
    return out.reshape(TKIN, 5 * TMOUT)


def _make_slabs(X16):
    """[8, SLAB, LEXT] fp16 slabs: staging cols 0..5 then X then 2 zero."""
    slabs = np.zeros((NCORES, SLAB, LEXT), dtype=np.float16)
    for c in range(NCORES):
        lo = c * RPC - 1
        hi = c * RPC + RPC + 1
        src_lo = max(0, lo)
        src_hi = min(H, hi)
        slabs[c, src_lo - lo : src_hi - lo, STG : STG + L] = X16[src_lo:src_hi, :]
    slabs[:, :, 0] = slabs[:, :, STG]          # X0
    slabs[:, :, 3] = slabs[:, :, STG + L - 1]  # XL
    return slabs


def _make_xtail(X16):
    """[8, TKIN, TXF] fp16 folded tail input, partition p = g*TR + r."""
    xt = np.zeros((NCORES, TKIN, TXF), dtype=np.float16)
    for c in range(NCORES):
        for r in range(TR):
            gr = c * RPC + MAIN - 1 + r
            if gr >= H:
                continue
            row = X16[gr]
            for g in range(TG):
                j0 = g * TGW - 1
                a = max(0, j0)
                b = min(L, j0 + TGW + 2)
                if a < b:
                    xt[c, g * TR + r, a - j0 : b - j0] = row[a:b]
            xt[c, 0 * TR + r, TGW + 2] = row[0]       # SL staging [X0, 0]
            xt[c, (TG - 1) * TR + r, TGW + 5] = row[L - 1]  # SR staging [0, XL]
    return xt


# ----------------------------------------------------------------------------
# Device program (SPMD; identical instruction stream on all 8 cores)
# ----------------------------------------------------------------------------

def build_nc(compile=True):
    nc = bacc.Bacc()
    x_d = nc.declare_dram_parameter("xslab", [SLAB, LEXT], F16, isOutput=False)
    sm_d = nc.declare_dram_parameter("smalls", [128, SM_TOT], F16, isOutput=False)
    y_d = nc.declare_dram_parameter("y", [MAIN, L], F16, isOutput=True)
    yt_d = nc.declare_dram_parameter("ytail", [TMOUT, TGW], F16, isOutput=True)

    with TileContext(nc) as tc:
        with (
            tc.tile_pool(name="wp", bufs=1) as wp,
            tc.tile_pool(name="xp", bufs=4) as xp,
            tc.tile_pool(name="yp", bufs=4) as yp,
            tc.tile_pool(name="pp", bufs=3, space="PSUM") as pp,
            tc.tile_pool(name="pt", bufs=1, space="PSUM") as ppt,
            tc.tile_pool(name="pw", bufs=1, space="PSUM") as ppw,
        ):
            smsb = wp.tile([128, SM_TOT], F16, name="smsb")
            wsb = smsb
            wz = wp.tile([128, 512], F16, name="wz")
            ytlsb = wp.tile([TMOUT, TGW], F16, name="ytlsb")

            # -- input DMA triggers, all up front (queues stream ahead) --
            # sync: packed smalls (one fat-line DMA), then later-block L halves
            nc.sync.dma_start(out=smsb[:, :], in_=sm_d[:, :])

            xts = [
                xp.tile([128, LEXT], F16, name=f"xt{b}", tag="xt")
                for b in range(NBLK)
            ]
            # L half covers staging + X cols up to pair-1's v=+1 reach, so
            # pairs 0,1 of a block depend only on the L piece
            HALF = STG + 2050
            # block 0 on scalar (L then R) so it races the smalls on sync
            nc.scalar.dma_start(out=xts[0][:, 0:HALF], in_=x_d[0:128, 0:HALF])
            nc.scalar.dma_start(
                out=xts[0][:, HALF:LEXT], in_=x_d[0:128, HALF:LEXT]
            )
            for b in range(1, NBLK):
                s = b * M
                nc.sync.dma_start(
                    out=xts[b][:, 0:HALF], in_=x_d[s : s + 128, 0:HALF]
                )
                nc.scalar.dma_start(
                    out=xts[b][:, HALF:LEXT], in_=x_d[s : s + 128, HALF:LEXT]
                )

            # -- PE warmup (p-state ramp); results are discarded --
            nc.vector.memset(wz[:, :], 0.0)
            pw = ppw.tile([128, 512], F32, name="pwarm")
            for i in range(NWARM):
                nc.tensor.matmul(
                    pw[0:128, 0:512], wz[0:128, 0:128], wz[0:128, 0:512],
                    start=True, stop=True,
                )

            def twm(w):
                c0 = SM_WS + w * TMOUT
                return smsb[0:TKIN, c0 : c0 + TMOUT]

            def txr(off, n):
                return smsb[0:TKIN, SM_WT + off : SM_WT + off + n]

            def do_tail():
                # column-folded tail; data rides in the packed smalls
                ptl = ppt.tile([128, TGW], F32, name="ptail")
                nc.tensor.matmul(
                    ptl[0:TMOUT, 0:TGW], twm(0), txr(0, TGW),
                    start=True, stop=False,
                )
                nc.tensor.matmul(
                    ptl[0:TMOUT, 0:TGW], twm(1), txr(1, TGW),
                    start=False, stop=False,
                )
                nc.tensor.matmul(
                    ptl[0:TMOUT, 0:TGW], twm(2), txr(2, TGW),
                    start=False, stop=False,
                )
                nc.tensor.matmul(
                    ptl[0:TMOUT, 0:2], twm(3), txr(TGW + 2, 2),
                    start=False, stop=False,
                )
                nc.tensor.matmul(
                    ptl[0:TMOUT, TGW - 10 : TGW - 8], twm(4), txr(TGW + 4, 2),
                    start=False, stop=True,
                )
                nc.vector.tensor_copy(ytlsb[0:TMOUT, 0:TGW], ptl[0:TMOUT, 0:TGW])
                nc.gpsimd.dma_start(out=yt_d[:, :], in_=ytlsb[0:TMOUT, 0:TGW])

            # -- main blocks (tail slots in after block 0) --
            for b in range(NBLK):
                s = b * M
                si = 0 if b == 0 else 1
                xt = xts[b]
                yt = yp.tile([128, L], F16, name=f"yt{b}", tag="yt")

                def wm(w):
                    base = (si * 5 + w) * WPAD_M
                    return wsb[0:128, base : base + M]

                def xr(c0, n):
                    # moving slice for X cols [c0, c0+n) (c0 may be -1 to
                    # touch the staged zero at tile col 5)
                    return xt[0:128, STG + c0 : STG + c0 + n]

                for k in range(NPAIR):
                    pt = pp.tile([128, PAIR], F32, name=f"pt{b}_{k}", tag="pt")
                    cA = k * PAIR
                    cB = cA + 512
                    # band v=0 (start clears each psum half)
                    nc.tensor.matmul(
                        pt[0:M, 0:512], wm(1), xr(cA, 512), start=True, stop=False
                    )
                    nc.tensor.matmul(
                        pt[0:M, 512:1024], wm(1), xr(cB, 512), start=True, stop=False
                    )
                    # band v=-1
                    nc.tensor.matmul(
                        pt[0:M, 0:512], wm(0), xr(cA - 1, 512),
                        start=False, stop=False,
                    )
                    nc.tensor.matmul(
                        pt[0:M, 512:1024], wm(0), xr(cB - 1, 512),
                        start=False, stop=False,
                    )
                    # band v=+1 (+ edge fixes on the outermost pairs)
                    nc.tensor.matmul(
                        pt[0:M, 0:512], wm(2), xr(cA + 1, 512),
                        start=False, stop=(k != 0),
                    )
                    if k == 0:
                        # col 0 fix: moving staging [X0, 0] at tile cols 0:2
                        nc.tensor.matmul(
                            pt[0:M, 0:2], wm(3), xt[0:128, 0:2],
                            start=False, stop=True,
                        )
                    nc.tensor.matmul(
                        pt[0:M, 512:1024], wm(2), xr(cB + 1, 512),
                        start=False, stop=(k != NPAIR - 1),
                    )
                    if k == NPAIR - 1:
                        # col L-1 fix: moving staging [0, XL] at tile cols 2:4
                        nc.tensor.matmul(
                            pt[0:M, 1022:1024], wm(4), xt[0:128, 2:4],
                            start=False, stop=True,
                        )
                    # evacuate pair to SBUF (fp32 -> fp16); the very last
                    # pair is split across both engines for tail latency
                    if b == NBLK - 1 and k == NPAIR - 1:
                        nc.scalar.copy(
                            yt[0:M, cA : cA + 512], pt[0:M, 0:512]
                        )
                        nc.vector.tensor_copy(
                            yt[0:M, cB : cB + 512], pt[0:M, 512:1024]
                        )
                    else:
                        dst = yt[0:M, cA : cA + PAIR]
                        src = pt[0:M, 0:PAIR]
                        if k % 2 == 0:
                            nc.scalar.copy(dst, src)
                        else:
                            nc.vector.tensor_copy(dst, src)
                    # output DMA per block at k==3 below
                    if False:
                        pass
                    elif k == 3:
                        # full-block output on the (warmed) hard queues,
                        # alternating; last block split across both queues
                        if b == 0:
                            nc.sync.dma_start(
                                out=y_d[s : s + M, 0:L], in_=yt[0:M, 0:L]
                            )
                        elif b == 1:
                            nc.scalar.dma_start(
                                out=y_d[s : s + M, 0:L], in_=yt[0:M, 0:L]
                            )
                        elif b == 2:
                            nc.sync.dma_start(
                                out=y_d[s : s + M, 0:L], in_=yt[0:M, 0:L]
                            )
                        else:
                            nc.scalar.dma_start(
                                out=y_d[s : s + M, 0:2048], in_=yt[0:M, 0:2048]
                            )
                            nc.sync.dma_start(
                                out=y_d[s : s + M, 2048:L], in_=yt[0:M, 2048:L]
                            )
                if b == 0:
                    do_tail()
    if compile:
        nc.compile()
    return nc


_NC_CACHE = None


def _get_nc():
    global _NC_CACHE
    if _NC_CACHE is None:
        _NC_CACHE = build_nc()
    return _NC_CACHE


def _run(X, W, trace=False, **spmd_kwargs):
    X16 = np.asarray(X, dtype=np.float16)
    slabs = _make_slabs(X16)
    xtails = _make_xtail(X16)
    in_maps = []
    for c in range(NCORES):
        in_maps.append(
            {
                "xslab": slabs[c],
                "smalls": _build_smalls(W, c, xtails[c]),
            }
        )
    res = run_bass_kernel_spmd(
        _get_nc(), in_maps, core_ids=list(range(NCORES)), trace=trace, **spmd_kwargs
    )
    Y = np.empty((H, L), dtype=np.float32)
    for c in range(NCORES):
        r0 = c * RPC
        Y[r0 : r0 + MAIN] = res.results[c]["y"].astype(np.float32)
        ytl = res.results[c]["ytail"].astype(np.float32)  # [96, 342]
        tail = ytl.reshape(TG, TM, TGW).transpose(1, 0, 2).reshape(TM, TG * TGW)
        Y[r0 + MAIN : r0 + RPC] = tail[:, :L]
    return Y, res


def kernel(X, W):
    Y, _ = _run(X, W)
    return Y
